# revision 15
# baseline (speedup 1.0000x reference)
"""Multi-head self-attention kernel for 8 Trainium2 NeuronCores.

Problem: B=2, S=2048, D=1024, H=16 heads, head_dim=64, fp32 in/out.
Sharding: core = (batch b, head-group g of 4 heads); b = core//4, g = core%4.
Each core computes its 4 heads' attention for its batch plus a partial
output projection (wo row-sharded); the host sums the 4 partials per batch
and adds the constant (bv @ wo + bo) row.

Device dataflow (transposed layout: contraction dim always on SBUF
partitions; matmuls in float32r — fp32 bits at 2 PE-cycles/row):
  xT [D,S] -> QT = wq^T xT + bq [256,S] (2 tiles, head pair per tile),
  KT likewise, V = x wv [S,256] -> V' tiles [128,4,65] (ones col fused)
  per head pair / q-tile of 512 / k-tile of 128:
    ST = K Q^T   (row-packed K=64 matmul pair -> one PSUM [128,1024])
    PT = exp(ST/8)   (ScalarE, scale fused, one inst per k-tile)
    CT' += V'^T PT   (row 64 = softmax denominator)
  CT = CT' * recip(denom)  (DVE; odd heads DMA-shifted to partitions 64-127
                            so CT packs into [128,S] per pair)
  out_partial = sum_pair CT_pair^T-contraction with wo[128 rows] -> [S,D]
"""
import numpy as np

import concourse.mybir as mybir
import concourse.tile as tile
from concourse import bacc
from concourse.bass_utils import run_bass_kernel_spmd

F32 = mybir.dt.float32
F32R = mybir.dt.float32r
BF16 = mybir.dt.bfloat16
EXP = mybir.ActivationFunctionType.Exp

S = 2048            # sequence length
D = 1024            # embed dim
HPC = 4             # heads per core
HD = 64             # head dim
GD = HPC * HD       # 256, per-core slice of D for QKV
NDK = D // 128      # 8 k-tiles over D
NKT = S // 128      # 16 k-tiles over S (attention contraction)
NQT = S // 512      # 4 q-tiles of 512

MMDT = F32R         # matmul dtype: F32R (accurate) or BF16 (fast)

_CACHED = {}


def _np_mm():
    if MMDT == BF16:
        import ml_dtypes
        return ml_dtypes.bfloat16
    return np.float32


def build_nc():
    mmdt = MMDT
    nc = bacc.Bacc("TRN2", target_bir_lowering=False, debug=False, num_devices=8)
    # host-prepped layouts: xT transposed; w* as [128, NDK*GD] (p, k, m);
    # biases as [128, 2] (p, m); wo natural [GD, D].
    xT = nc.dram_tensor("xT", [D, S], mmdt, kind="ExternalInput").ap()
    wq = nc.dram_tensor("wq", [128, NDK * GD], mmdt, kind="ExternalInput").ap()
    wk = nc.dram_tensor("wk", [128, NDK * GD], mmdt, kind="ExternalInput").ap()
    wv = nc.dram_tensor("wv", [128, NDK * GD], mmdt, kind="ExternalInput").ap()
    wo = nc.dram_tensor("wo", [GD, D], mmdt, kind="ExternalInput").ap()
    bq = nc.dram_tensor("bq", [128, 2], F32, kind="ExternalInput").ap()
    bk = nc.dram_tensor("bk", [128, 2], F32, kind="ExternalInput").ap()
    out = nc.dram_tensor("out", [S, D], F32, kind="ExternalOutput").ap()

    with tile.TileContext(nc) as tc:
        with tc.tile_pool(name="persist", bufs=1) as pw, \
             tc.tile_pool(name="projx", bufs=NDK) as pjx, \
             tc.tile_pool(name="projw", bufs=1) as pjw, \
             tc.tile_pool(name="scratch", bufs=2, space="PSUM") as scratch, \
             tc.tile_pool(name="stps", bufs=2, space="PSUM") as stps, \
             tc.tile_pool(name="ctps", bufs=1, space="PSUM") as ctps, \
             tc.tile_pool(name="ptp", bufs=3) as ptp, \
             tc.tile_pool(name="smalls", bufs=2) as smalls:

            # ---- long-lived tensors -------------------------------------
            qt_sb = [pw.tile([128, S], mmdt, tag=f"qt{m}", name=f"qt{m}")
                     for m in range(2)]
            kt_sb = [pw.tile([128, S], mmdt, tag=f"kt{m}", name=f"kt{m}")
                     for m in range(2)]
            vp_sb = [pw.tile([128, HPC, HD + 1], mmdt, tag=f"vp{s}",
                             name=f"vp{s}") for s in range(NKT)]
            ctp_sb = [pw.tile([128, S], mmdt, tag=f"ctp{p}", name=f"ctp{p}")
                      for p in range(2)]
            wop_sb = [pw.tile([128, D], mmdt, tag=f"wop{p}", name=f"wop{p}")
                      for p in range(2)]

            # ---- input DMAs (xT + pair-0 weights first) -----------------
            wq_sb = pjw.tile([128, NDK, GD], mmdt, tag="wq")
            wk_sb = pjw.tile([128, NDK, GD], mmdt, tag="wk")
            wv_sb = pjw.tile([128, NDK, GD], mmdt, tag="wv")
            bq_sb = pjw.tile([128, 2], F32, tag="bq")
            bk_sb = pjw.tile([128, 2], F32, tag="bk")
            ones_t = pjw.tile([128, 1], F32, tag="ones")
            wq_v = wq.rearrange("p (k m) -> p k m", k=NDK)
            nc.sync.dma_start(wq_sb[:, 0:2], wq_v[:, 0:2])
            nc.sync.dma_start(bq_sb[:], bq)
            nc.sync.dma_start(wq_sb[:, 2:NDK], wq_v[:, 2:NDK])
            x_sb = []
            for k in range(NDK):
                xt = pjx.tile([128, S], mmdt, tag="xT", name=f"x{k}")
                for h in range(2):
                    nc.sync.dma_start(xt[:, 1024 * h:1024 * h + 1024],
                                      xT[128 * k:128 * k + 128,
                                         1024 * h:1024 * h + 1024])
                x_sb.append(xt)
            nc.sync.dma_start(wk_sb[:], wk.rearrange("p (k m) -> p k m", k=NDK))
            nc.sync.dma_start(bk_sb[:], bk)
            nc.sync.dma_start(wv_sb[:], wv.rearrange("p (k m) -> p k m", k=NDK))
            for p in range(2):
                nc.sync.dma_start(wop_sb[p][:], wo[128 * p:128 * p + 128, :])
            nc.vector.memset(ones_t[:], 1.0)

            def proj_qk(w_sb, b_sb, dst, m):
                """dst[m] [128, S] = (w slice)^T xT + bias."""
                for n in range(4):
                    ps = scratch.tile([128, 512], F32, tag="sc", name="psqk")
                    for k in range(NDK):
                        nc.tensor.matmul(
                            ps[:], w_sb[:, k, 128 * m:128 * m + 128],
                            x_sb[k][:, 512 * n:512 * n + 512],
                            start=(k == 0), stop=(k == NDK - 1))
                    nc.vector.tensor_scalar_add(
                        dst[m][:, 512 * n:512 * n + 512],
                        ps[:], b_sb[:, m:m + 1])

            def proj_v(s):
                """V' tile s: [128, HPC, 65] with ones col."""
                vt = scratch.tile([128, GD], F32, tag="sc", name="v")
                for k in range(NDK):
                    nc.tensor.matmul(
                        vt[:], x_sb[k][:, 128 * s:128 * s + 128],
                        wv_sb[:, k, :],
                        start=(k == 0), stop=(k == NDK - 1))
                nc.vector.tensor_copy(
                    vp_sb[s][:, :, 0:64],
                    vt[:].rearrange("p (h d) -> p h d", h=HPC))
                nc.vector.tensor_copy(
                    vp_sb[s][:, :, 64:65],
                    ones_t[:, None, :].broadcast_to([128, HPC, 1]))

            def emit_pv(pair, kt, ct_ps, pts):
                pt = pts.pop(kt)
                for par in range(2):
                    nc.tensor.matmul(
                        ct_ps[par][:, :],
                        vp_sb[kt][:, 2 * pair + par, :],
                        pt[:, 512 * par:512 * par + 512],
                        start=(kt == 0), stop=(kt == NKT - 1))

            def attention(pair, qt, interleave_v=False):
                q0 = 512 * qt
                ct_ps = [ctps.tile([65, 512], F32, tag=f"ctp{par}",
                                   name=f"ctps{par}") for par in range(2)]
                pts = {}
                for kt in range(NKT):
                    st = stps.tile([128, 1024], F32, tag="st", name="st")
                    for par in range(2):
                        p0 = 64 * par
                        nc.tensor.matmul(
                            st[:, 512 * par:512 * par + 512],
                            kt_sb[pair][p0:p0 + 64, 128 * kt:128 * kt + 128],
                            qt_sb[pair][p0:p0 + 64, q0:q0 + 512],
                            start=True, stop=True, tile_position=(p0, 0))
                    pt = ptp.tile([128, 1024], mmdt, tag="pt", name="pt")
                    nc.scalar.activation(pt[:], st[:], EXP, scale=0.125)
                    pts[kt] = pt
                    if interleave_v:
                        proj_v(kt)
                    if kt > 0:
                        emit_pv(pair, kt - 1, ct_ps, pts)
                emit_pv(pair, NKT - 1, ct_ps, pts)

                for par in range(2):
                    # ScalarE (idle at this point) copies ct out, releasing
                    # the PSUM bank fast; normalization then runs on DVE off
                    # the PE critical path
                    ctsb = smalls.tile([65, 512], F32, tag="ctsb",
                                       name="ctsb")
                    nc.scalar.copy(ctsb[:], ct_ps[par][:])
                    with tc.high_priority(-300):
                        # negative offset = LOWER priority: scheduler places
                        # the normalize chain after pending evacuation copies
                        # so it cannot head-of-line-block the DVE queue
                        rrec = smalls.tile([1, 512], F32, tag="rrec",
                                           name="rrec")
                        nc.vector.reciprocal(rrec[:], ctsb[64:65, :])
                        rb = smalls.tile([64, 512], F32, tag="rb", name="rb")
                        nc.gpsimd.partition_broadcast(rb[:], rrec[:])
                        if par == 0:
                            nc.vector.tensor_mul(
                                ctp_sb[pair][0:64, q0:q0 + 512],
                                ctsb[0:64, :], rb[:])
                        else:
                            todd = smalls.tile([64, 512], mmdt, tag="todd",
                                               name="todd")
                            nc.vector.tensor_mul(todd[:], ctsb[0:64, :],
                                                 rb[:])
                            nc.sync.dma_start(
                                ctp_sb[pair][64:128, q0:q0 + 512], todd[:])

            def outproj(qt):
                for si in range(4 * qt, 4 * qt + 4):
                    osb = smalls.tile([128, 1024], F32, tag="osb", name="osb")
                    for n in range(2):
                        op = scratch.tile([128, 512], F32, tag="sc", name="op")
                        for p in range(2):
                            nc.tensor.matmul(
                                op[:],
                                ctp_sb[p][:, 128 * si:128 * si + 128],
                                wop_sb[p][:, 512 * n:512 * n + 512],
                                start=(p == 0), stop=(p == 1))
                        nc.vector.tensor_copy(osb[:, 512 * n:512 * n + 512],
                                              op[:])
                    nc.sync.dma_start(out[128 * si:128 * si + 128, :], osb[:])

            # ---- emission schedule --------------------------------------
            proj_qk(wq_sb, bq_sb, qt_sb, 0)
            proj_qk(wk_sb, bk_sb, kt_sb, 0)
            attention(0, 0, interleave_v=True)
            proj_qk(wq_sb, bq_sb, qt_sb, 1)
            proj_qk(wk_sb, bk_sb, kt_sb, 1)
            attention(1, 0)
            for qt in range(1, NQT):
                attention(0, qt)
                outproj(qt - 1)
                attention(1, qt)
            outproj(NQT - 1)


    nc.compile()
    return nc


def _get_nc():
    if MMDT not in _CACHED:
        _CACHED[MMDT] = build_nc()
    return _CACHED[MMDT]


def shard_inputs(x, wq, bq, wk, bk, wv, bv, wo, bo):
    np_mm = _np_mm()
    x = np.asarray(x, dtype=np.float32)
    wq, bq = np.asarray(wq, np.float32), np.asarray(bq, np.float32)
    wk, bk = np.asarray(wk, np.float32), np.asarray(bk, np.float32)
    wv = np.asarray(wv, np.float32)
    wo = np.asarray(wo, np.float32)

    def wlayout(w):  # [D, GD] -> [128, NDK*GD] with (p, k, m) order
        return np.ascontiguousarray(
            w.reshape(NDK, 128, GD).transpose(1, 0, 2).reshape(128, NDK * GD)
        ).astype(np_mm)

    def blayout(b):  # [GD] -> [128, 2] with (p, m) order
        return np.ascontiguousarray(b.reshape(2, 128).T).astype(np.float32)

    in_maps = []
    for core in range(8):
        b, g = core // 4, core % 4
        hs = slice(g * GD, (g + 1) * GD)
        in_maps.append({
            "xT": np.ascontiguousarray(x[b].T).astype(np_mm),
            "wq": wlayout(wq[:, hs]),
            "wk": wlayout(wk[:, hs]),
            "wv": wlayout(wv[:, hs]),
            "wo": np.ascontiguousarray(wo[hs, :]).astype(np_mm),
            "bq": blayout(bq[hs]),
            "bk": blayout(bk[hs]),
        })
    return in_maps


def kernel(x, wq, bq, wk, bk, wv, bv, wo, bo, _trace=False, _tracekw=None):
    nc = _get_nc()
    in_maps = shard_inputs(x, wq, bq, wk, bk, wv, bv, wo, bo)
    kw = dict(_tracekw or {})
    res = run_bass_kernel_spmd(nc, in_maps, core_ids=list(range(8)),
                               trace=_trace, **kw)
    kernel._last_result = res
    # constant row: bv @ wo + bo (exact in float64, folded on host)
    bv64 = np.asarray(bv, np.float64)
    wo64 = np.asarray(wo, np.float64)
    bo64 = np.asarray(bo, np.float64)
    const_row = bv64 @ wo64 + bo64
    full = np.empty((2, S, D), np.float32)
    for b in range(2):
        acc = np.zeros((S, D), np.float64)
        for g in range(4):
            acc += res.results[4 * b + g]["out"]
        full[b] = (acc + const_row).astype(np.float32)
    return full


kernel._last_result = None
